# revision 1
# baseline (speedup 1.0000x reference)
"""Trainium2 Bass kernel for nn_LinearKAN (histogram_binning).

Math
----
reference computes, per (batch b, out o):

    out[b,o] = sum_i  PL_interp(x[b,i]; bp[o,i,:], val[o,i,:])

where bp is the SAME sorted uniform grid for every (o,i) (tiled
linspace).  With u = (x - bp0)/h in [0, S), the piecewise-linear
interpolant has an exact *clamp basis* expansion

    f(u) = val_0 + sum_{s=0..S-1} M_s * clamp(u - s, 0, 1)
    M_s  = val_{s+1} - val_s              (segment slopes)

so the layer is a bias plus S dense matmuls contracting over (s, i):

    out[b,o] = bias[o] + sum_s sum_i M_s[o,i] * r_s[b,i]
    r_s      = clamp(u - s, 0, 1),   bias[o] = sum_i val[o,i,0]

The clamp basis quantizes benignly: r entries are exactly 0, exactly 1,
or the single fractional t per (b,i) -- so fp16 operands lose almost
nothing.  The slopes are split M = M_hi + M_lo/2048 with both parts
fp16 (2048 scaling keeps M_lo out of fp16-denormal range), accumulated
into two PSUM groups and combined in the tail:
out = ps_hi + 2^-11 * ps_lo + bias.  Measured ~2e-4 rel err.

Device kernel (per core, SPMD over 8 cores):
  - shard batch into 4 quarters (B_loc=256) x out-features into 2
    halves (O_loc=128); no cross-device reduction.
  - u^T [i, (ih,b)] via one ScalarE activation; r_s tiles [128, 512]
    fp16 via VectorE/ScalarE (relu then min-1); 2x40 fp16 matmuls
    (K=128 chunks of the (s,i) contraction) at full PE rate; tail
    DVE combine + bias; DMA out.
Host only slices/transposes/differences the params (layout prep).
"""

import os
import numpy as np

import concourse.bass as bass
import concourse.mybir as mybir
import concourse.tile as tile
from concourse import bacc
from concourse.bass_utils import run_bass_kernel_spmd

# Problem shape (hardcoded per the task contract).
B, O, I, S = 1024, 256, 256, 20
N_CORES = 8
B_SPLIT, O_SPLIT = 4, 2
B_LOC, O_LOC = B // B_SPLIT, O // O_SPLIT  # 256, 128
KT = 2 * S          # 40 K-tiles of 128 over the (s, i) contraction
CHUNK_KT = (4, 6, 8, 10, 12)  # C DMA chunk sizes in kt (smallest first)
LO_SCALE = 2048.0   # M_lo pre-scale (keeps fp16 normal); undone in tail
F32 = mybir.dt.float32
F16 = mybir.dt.float16
FW = 2 * B_LOC      # r/u tile free width: both i-halves side by side

# s values whose relu step runs on ScalarE (rest on VectorE); the min-1
# step always runs on VectorE.
ACT_RELU_S = set(range(8, 20))
N_WARMUP_MM = int(os.environ.get("KAN_WARMUP", "10"))  # PE HAM warmup dummies
N_GPS = int(os.environ.get("KAN_GPS", "0"))  # s-values built on GpSimd


def _strip_init_boilerplate(nc) -> None:
    """Drop the Bass-init const-AP memsets + all-engine barrier (~1.5us of
    preamble).  This kernel never reads the const APs (all activation biases
    are explicit APs), so the memsets and their barrier are dead weight."""
    blk = nc.m.functions[0].blocks[0]
    drop = (mybir.InstMemset, mybir.InstDrain, mybir.InstEventSemaphore)
    keep = [i for i in blk.instructions if not isinstance(i, drop)]
    del blk.instructions[:]
    for i in keep:
        blk.instructions.append(i)
    nc.const_aps.aps.clear()


def _build_nc(scale: float, ubias: float) -> bass.Bass:
    """Build the (SPMD-identical) single-core Bass graph."""
    nc = bacc.Bacc("TRN2", target_bir_lowering=False, debug=False)
    _strip_init_boilerplate(nc)

    xT = nc.declare_dram_parameter("xT", [128, FW], F32, isOutput=False)
    C2 = nc.declare_dram_parameter("C2", [128, 2 * KT * 128], F16,
                                   isOutput=False)
    bias0 = nc.declare_dram_parameter("bias0", [128, 1], F32, isOutput=False)
    out = nc.declare_dram_parameter("out", [O_LOC, B_LOC], F32, isOutput=True)

    with tile.TileContext(nc) as tc:
        with (
            tc.tile_pool(name="xt", bufs=1) as xpool,
            tc.tile_pool(name="u", bufs=1) as upool,
            tc.tile_pool(name="w", bufs=4) as wpool,
            tc.tile_pool(name="wact", bufs=len(ACT_RELU_S)) as wapool,
            tc.tile_pool(name="r", bufs=S) as rpool,
            tc.tile_pool(name="c", bufs=1) as cpool,
            tc.tile_pool(name="b", bufs=1) as bpool,
            tc.tile_pool(name="o", bufs=2) as opool,
            tc.tile_pool(name="ps", bufs=2, space="PSUM") as pspool,
        ):
            # --- ACT bias-constant table via gpsimd memsets (no DMA dep):
            # col 1 = ubias, col 1+s = -s for the ScalarE-assigned s.
            ctab = bpool.tile([128, 24], F32, tag="ctab")
            nc.gpsimd.memset(ctab[:, 1:2], float(ubias))
            for s in range(1, S):
                if s in ACT_RELU_S or s == S - 1:
                    nc.gpsimd.memset(ctab[:, 1 + s:2 + s], -float(s))

            # --- PE HAM warmup: dummy matmuls on memset scratch so the
            # clock-gate opens (1.2 -> 2.4 GHz) before the real stream.
            if N_WARMUP_MM:
                wa = wpool.tile([128, 128], F16, tag="warm_a")
                wb = wpool.tile([128, 512], F16, tag="warm_b")
                nc.gpsimd.memset(wa[:], 0.0)
                nc.gpsimd.memset(wb[:], 0.0)
                ps_warm = pspool.tile([128, 512], F32, tag="pw")
                for _ in range(N_WARMUP_MM):
                    nc.tensor.matmul(ps_warm[:], wa[:], wb[:],
                                     start=True, stop=True)

            # ACT table preload: cheap Copy on the memset const table.
            dummy = wpool.tile([128, 1], F32, tag="dummy")
            nc.scalar.copy(dummy[:], ctab[:, 1:2])

            # --- DMA in, all on the sync HWDGE queue; order matters:
            # xT first (it gates the whole ACT/DVE production chain),
            # then C chunks smallest-first; bias0 (tail-only) last.
            xt = xpool.tile([128, FW], F32)
            nc.sync.dma_start(xt[:], xT[:])
            chi = {}
            clo = {}
            kt0 = 0
            for ci, nkt in enumerate(CHUNK_KT):
                t = cpool.tile([128, nkt * 256], F16, tag=f"c{ci}")
                nc.sync.dma_start(
                    t[:], C2[:, kt0 * 256:(kt0 + nkt) * 256])
                for k in range(nkt):
                    chi[kt0 + k] = t[:, k * 128:(k + 1) * 128]
                    clo[kt0 + k] = t[:, (nkt + k) * 128:(nkt + k + 1) * 128]
                kt0 += nkt
            bias0_sb = bpool.tile([128, 1], F32, tag="b0")
            nc.sync.dma_start(bias0_sb[:], bias0[:])

            # --- u = relu(scale*x + ubias), one op over both i-halves ---
            u2 = upool.tile([128, FW], F32)
            nc.scalar.activation(
                u2[:], xt[:], mybir.ActivationFunctionType.Relu,
                bias=ctab[:, 1:2], scale=float(scale),
            )

            # --- r_s = clamp(u - s, 0, 1) in fp16 ---
            r = []
            for s in range(S):
                rs = rpool.tile([128, FW], F16, tag="r")
                if s == 0:
                    # u >= 0, so clamp(u,0,1) = min(u,1)
                    nc.vector.tensor_scalar(
                        rs[:], u2[:], 1.0, None, mybir.AluOpType.min)
                elif s == S - 1:
                    # u < 20, so clamp(u-19,0,1) = relu(u-19)
                    nc.scalar.activation(
                        rs[:], u2[:], mybir.ActivationFunctionType.Relu,
                        bias=ctab[:, 1 + s:2 + s], scale=1.0)
                elif s in ACT_RELU_S:
                    # fp16 intermediate: values >= 1 still clamp to exactly
                    # 1.0 after quantization, t-entries keep fp16 precision,
                    # and the 16-bit input speeds up the DVE min.
                    w = wapool.tile([128, FW], F16, tag="w_act")
                    nc.scalar.activation(
                        w[:], u2[:], mybir.ActivationFunctionType.Relu,
                        bias=ctab[:, 1 + s:2 + s], scale=1.0)
                    nc.vector.tensor_scalar(
                        rs[:], w[:], 1.0, None, mybir.AluOpType.min)
                elif s <= N_GPS:
                    w = wpool.tile([128, FW], F16, tag="w_gps")
                    nc.gpsimd.tensor_scalar(
                        w[:], u2[:], float(s), float(s),
                        mybir.AluOpType.max, mybir.AluOpType.subtract)
                    nc.gpsimd.tensor_scalar(
                        rs[:], w[:], 1.0, None, mybir.AluOpType.min)
                else:
                    w = wpool.tile([128, FW], F16, tag="w_dve")
                    nc.vector.tensor_scalar(
                        w[:], u2[:], float(s), float(s),
                        mybir.AluOpType.max, mybir.AluOpType.subtract)
                    nc.vector.tensor_scalar(
                        rs[:], w[:], 1.0, None, mybir.AluOpType.min)
                r.append(rs)

            # --- matmuls: hi/lo interleaved per kt, two PSUM groups ---
            ps_hi = pspool.tile([O_LOC, B_LOC], F32, tag="ph")
            ps_lo = pspool.tile([O_LOC, B_LOC], F32, tag="pl")
            if os.environ.get("KAN_INTERLEAVE", "1") == "1":
                for kt in range(KT):
                    s, ih = kt // 2, kt % 2
                    rhs = r[s][:, ih * B_LOC:(ih + 1) * B_LOC]
                    nc.tensor.matmul(ps_hi[:], chi[kt], rhs,
                                     start=(kt == 0), stop=(kt == KT - 1))
                    nc.tensor.matmul(ps_lo[:], clo[kt], rhs,
                                     start=(kt == 0), stop=(kt == KT - 1))
            else:
                for ps, carr in ((ps_hi, chi), (ps_lo, clo)):
                    for kt in range(KT):
                        s, ih = kt // 2, kt % 2
                        rhs = r[s][:, ih * B_LOC:(ih + 1) * B_LOC]
                        nc.tensor.matmul(ps[:], carr[kt], rhs,
                                         start=(kt == 0), stop=(kt == KT - 1))

            # --- tail: out = ps_hi + ps_lo/2048 + bias ---
            t1 = opool.tile([O_LOC, B_LOC], F32, tag="t1")
            nc.vector.tensor_scalar(
                t1[:], ps_lo[:], 1.0 / LO_SCALE, bias0_sb[:, 0:1],
                mybir.AluOpType.mult, mybir.AluOpType.add)
            out_sb = opool.tile([O_LOC, B_LOC], F32, tag="osb")
            nc.vector.tensor_tensor(
                out_sb[:], ps_hi[:], t1[:], mybir.AluOpType.add)
            nc.sync.dma_start(out[:], out_sb[:])
    nc.compile()
    return nc


_NC_CACHE: dict = {}


def _get_nc(scale: float, ubias: float) -> bass.Bass:
    key = (float(scale), float(ubias))
    if key not in _NC_CACHE:
        _NC_CACHE[key] = _build_nc(scale, ubias)
    return _NC_CACHE[key]


def prepare(x: np.ndarray, breakpoints: np.ndarray, values: np.ndarray):
    """Host prep: build the Bass graph (cached) + per-core input maps."""
    x = np.asarray(x, np.float32)
    breakpoints = np.asarray(breakpoints, np.float32)
    values = np.asarray(values, np.float32)

    # Grid affine params from the (shared) breakpoint row.
    bpr = breakpoints[0, 0].astype(np.float64)
    h = (bpr[-1] - bpr[0]) / S
    scale = float(1.0 / h)
    ubias = float(-bpr[0] / h)

    # Clamp-basis slopes, split into fp16 hi + scaled fp16 lo.
    Vf = values  # [O, I, S+1]
    M = (Vf[:, :, 1:] - Vf[:, :, :-1]).transpose(2, 0, 1)  # [S, O, I] f32
    M = np.ascontiguousarray(M, np.float32)
    Mhi = M.astype(np.float16)
    Mlo = ((M - Mhi.astype(np.float32)) * LO_SCALE).astype(np.float16)
    bias_o = Vf[:, :, 0].sum(axis=1, dtype=np.float64).astype(np.float32)

    # Per-core layouts.
    #   C*: [j, kt, o] fp16 with kt = 2*s + ih, j = i within half.
    #   xT: [j, ih*B_LOC + b] fp32.
    Mhi_r = Mhi.reshape(S, O_SPLIT, O_LOC, 2, 128)  # [s, oh, o, ih, j]
    Mlo_r = Mlo.reshape(S, O_SPLIT, O_LOC, 2, 128)
    xr = x.reshape(B_SPLIT, B_LOC, 2, 128)          # [bq, b, ih, j]

    in_maps = []
    for c in range(N_CORES):
        bq, oh = c % B_SPLIT, c // B_SPLIT
        # xr[bq] axes (b, ih, j) -> (j, ih, b) -> [128, FW]
        xT_c = np.ascontiguousarray(
            xr[bq].transpose(2, 1, 0)).reshape(128, FW)
        C_hi = np.ascontiguousarray(
            Mhi_r[:, oh].transpose(3, 0, 2, 1)).reshape(128, KT * 128)
        C_lo = np.ascontiguousarray(
            Mlo_r[:, oh].transpose(3, 0, 2, 1)).reshape(128, KT * 128)
        # Interleave hi/lo per DMA chunk: [hi kts of chunk][lo kts of chunk]
        blocks = []
        kt0 = 0
        for nkt in CHUNK_KT:
            blocks.append(C_hi[:, kt0 * 128:(kt0 + nkt) * 128])
            blocks.append(C_lo[:, kt0 * 128:(kt0 + nkt) * 128])
            kt0 += nkt
        C2_c = np.ascontiguousarray(np.concatenate(blocks, axis=1))
        b0 = np.ascontiguousarray(
            bias_o[oh * O_LOC:(oh + 1) * O_LOC].reshape(128, 1))
        in_maps.append({"xT": xT_c, "C2": C2_c, "bias0": b0})

    nc = _get_nc(scale, ubias)
    return nc, in_maps


def kernel(x: np.ndarray, breakpoints: np.ndarray, values: np.ndarray,
           **_extra) -> np.ndarray:
    nc, in_maps = prepare(x, breakpoints, values)
    res = run_bass_kernel_spmd(nc, in_maps, list(range(N_CORES)))

    outf = np.empty((B, O), np.float32)
    for c in range(N_CORES):
        bq, oh = c % B_SPLIT, c // B_SPLIT
        outf[bq * B_LOC:(bq + 1) * B_LOC, oh * O_LOC:(oh + 1) * O_LOC] = \
            res.results[c]["out"].T
    return outf


if __name__ == "__main__":
    rng = np.random.default_rng(0)
    x = rng.uniform(-1, 1, (B, I)).astype(np.float32)
    bp = np.tile(np.linspace(-1, 1, S + 1, dtype=np.float32), (O, I, 1))
    v = (rng.standard_normal((O, I, S + 1)) * 0.1).astype(np.float32)
    out = kernel(x, bp, v)
    print("kernel ran, out:", out.shape, out.dtype, float(out.std()))



# revision 6
# speedup vs baseline: 1.3146x; 1.3146x over previous
"""Trainium2 Bass kernel for nn_LinearKAN (histogram_binning), v2.

Math
----
reference computes, per (batch b, out o):

    out[b,o] = sum_i  PL_interp(x[b,i]; bp[o,i,:], val[o,i,:])

bp is the SAME sorted uniform grid for every (o,i) (tiled linspace).
With u = (x - bp0)/h in [0, S) and uc = u - S/2 in [-10, 10), any
continuous piecewise-linear function on the uniform grid has an exact
*two-sided kink basis* expansion (one ReLU kink per interior knot,
negative-side kinks folded into the affine part):

    f(uc) = a + b*uc + sum_{s'=1..9}  c_{s'} * (max(uc,s')-s')
                     + sum_{s'=-9..-1} d_{s'} * (min(uc,s')-s')

where c/d are second differences of the values (slope changes).  Each
basis tile is ONE tensor_scalar op (max/min then subtract) -- no clamp,
so half the elementwise work of the clamp basis -- and the layer is a
bias plus 20 dense K=128 matmul tiles contracting over (s', i):

    out[b,o] = bias[o] + sum_{s'} sum_i C_{s'}[o,i] * g_{s'}[b,i]

All operands fp16 (basis magnitudes <= 10 by the two-sided centering);
measured ~7e-3 rel err vs the 2e-2 gate.

Device kernel (per core, SPMD over 8 cores, 4 b-quarters x 2 o-halves):
  - ScalarE HWDGE queue DMAs xb (x fp16 + bias col); SyncE queue DMAs
    the C coefficients in 2 chunks; descriptor gen runs in parallel.
  - DVE: uc = scale*x (one 4x fp16 tensor_scalar), then most g tiles
    (194ns each at 4x); ScalarE Relu produces a few g tiles in between.
  - PE: HAM warmup dummies, then 40 fp16 matmuls (K=128 chunks of the
    (s',i) contraction) accumulating one PSUM group.
  - tail: ACT Identity adds bias + casts to fp16, DMA out.
Host only slices/transposes/differences the params (layout prep) and
casts dtypes.
"""

import os
import numpy as np

import concourse.bass as bass
import concourse.mybir as mybir
import concourse.tile as tile
from concourse import bacc
from concourse.bass_utils import run_bass_kernel_spmd

# Problem shape (hardcoded per the task contract).
B, O, I, S = 1024, 256, 256, 20
N_CORES = 8
B_SPLIT, O_SPLIT = 4, 2
B_LOC, O_LOC = B // B_SPLIT, O // O_SPLIT  # 256, 128
SC = S // 2         # grid center; uc = u - SC in [-10, 10)
NT = S              # basis tiles: uc (linear) + 19 kinks
KT = 2 * S          # 40 k-tiles of 128 over the (s', i) contraction
F32 = mybir.dt.float32
F16 = mybir.dt.float16
FW = 2 * B_LOC      # g/uc tile free width: both i-halves side by side
XB_COLS = FW + 2    # xb layout: [x data | bias col | pad]

N_WARMUP_MM = int(os.environ.get("KAN_WARMUP", "4"))  # PE HAM warmup dummies
N_ACT = int(os.environ.get("KAN_NACT", "5"))          # g tiles built on ACT
# C DMA chunk split (in tiles of the production ORDER; 2 kt per tile)
CHUNK_T = int(os.environ.get("KAN_CHUNK", "9"))

# Production/consumption order of basis tiles: uc first, then kinks by
# |s'|.  ACT-assigned tiles sit where their (later) completion lands.
_KINKS = [0]
for m in range(1, SC):
    _KINKS.append(m)
    _KINKS.append(-m)
# positions (0-based among the 19 kinks) handled by ACT
_ACT_POS = {5, 8, 11, 14, 17} if N_ACT == 5 else \
    set(np.linspace(4, 18, max(N_ACT, 1), dtype=int).tolist() if N_ACT else [])
ORDER = [(sp, (j in _ACT_POS)) for j, sp in enumerate(_KINKS)]  # (s', on_act)


def _strip_init_boilerplate(nc) -> None:
    """Drop the Bass-init const-AP memsets + all-engine barrier (~1.5us of
    preamble).  This kernel never reads the const APs (all activation biases
    are explicit APs), so the memsets and their barrier are dead weight."""
    blk = nc.m.functions[0].blocks[0]
    drop = (mybir.InstMemset, mybir.InstDrain, mybir.InstEventSemaphore)
    keep = [i for i in blk.instructions if not isinstance(i, drop)]
    del blk.instructions[:]
    for i in keep:
        blk.instructions.append(i)
    nc.const_aps.aps.clear()


def _build_nc(scale: float, ucbias: float) -> bass.Bass:
    """Build the (SPMD-identical) single-core Bass graph."""
    nc = bacc.Bacc("TRN2", target_bir_lowering=False, debug=False)
    _strip_init_boilerplate(nc)

    xb = nc.declare_dram_parameter("xb", [128, XB_COLS], F16, isOutput=False)
    C = nc.declare_dram_parameter("C", [128, KT * 128], F16, isOutput=False)
    out = nc.declare_dram_parameter("out", [O_LOC, B_LOC], F16, isOutput=True)

    with tile.TileContext(nc) as tc:
        with (
            tc.tile_pool(name="xb", bufs=1) as xpool,
            tc.tile_pool(name="u", bufs=1) as upool,
            tc.tile_pool(name="w", bufs=3) as wpool,
            tc.tile_pool(name="g", bufs=NT) as gpool,
            tc.tile_pool(name="c", bufs=2) as cpool,
            tc.tile_pool(name="b", bufs=1) as bpool,
            tc.tile_pool(name="o", bufs=1) as opool,
            tc.tile_pool(name="ps", bufs=2, space="PSUM") as pspool,
        ):
            # --- input DMAs, parallel descriptor-gen on two queues:
            # xb on the ScalarE HWDGE queue (gates the whole DVE chain),
            # C chunks on the SyncE queue.
            xb_sb = xpool.tile([128, XB_COLS], F16)
            nc.scalar.dma_start(xb_sb[:], xb[:])
            n1 = 2 * CHUNK_T + 2          # kts in chunk 1 (incl. uc's 2)
            c0 = cpool.tile([128, n1 * 128], F16, tag="c0")
            c1 = cpool.tile([128, (KT - n1) * 128], F16, tag="c1")
            nc.sync.dma_start(c0[:], C[:, : n1 * 128])
            nc.sync.dma_start(c1[:], C[:, n1 * 128:])
            ckt = {}
            for k in range(KT):
                if k < n1:
                    ckt[k] = c0[:, k * 128:(k + 1) * 128]
                else:
                    kk = k - n1
                    ckt[k] = c1[:, kk * 128:(kk + 1) * 128]

            # --- DVE preamble: warmup scratch + ACT bias-constant table
            # (no DMA dep, so PE warmup can start right after).
            wa = wpool.tile([128, 128], F16, tag="warm_a")
            wb = wpool.tile([128, 512], F16, tag="warm_b")
            nc.vector.memset(wa[:], 0.0)
            nc.vector.memset(wb[:], 0.0)
            ctab = bpool.tile([128, NT], F32, tag="ctab")
            act_bias_col = {}
            for j, (sp, on_act) in enumerate(ORDER):
                if on_act:
                    # relu(uc - s') for s'>0 : bias -s'; relu(s'-uc) for
                    # s'<0 (scale=-1): bias +s'
                    val = -float(sp) if sp >= 0 else float(sp)
                    nc.vector.memset(ctab[:, j:j + 1], val)
                    act_bias_col[j] = ctab[:, j:j + 1]

            # --- PE HAM warmup: dummy matmuls so the clock-gate opens
            # (1.2 -> 2.4 GHz) before the real stream.
            if N_WARMUP_MM:
                ps_warm = pspool.tile([128, 512], F32, tag="pw")
                for _ in range(N_WARMUP_MM):
                    nc.tensor.matmul(ps_warm[:], wa[:], wb[:],
                                     start=True, stop=True)

            # --- uc = scale*x + ucbias on DVE (4x fp16), both i-halves ---
            uc = upool.tile([128, FW], F16)
            nc.vector.tensor_scalar(
                uc[:], xb_sb[:, 0:FW], float(scale), float(ucbias),
                mybir.AluOpType.mult, mybir.AluOpType.add)

            # --- g tiles: one tensor_scalar / activation each ---
            g = [uc]
            for j, (sp, on_act) in enumerate(ORDER):
                gt = gpool.tile([128, FW], F16, tag="g")
                if on_act:
                    if sp >= 0:
                        nc.scalar.activation(
                            gt[:], uc[:], mybir.ActivationFunctionType.Relu,
                            bias=act_bias_col[j], scale=1.0)
                    else:
                        nc.scalar.activation(
                            gt[:], uc[:], mybir.ActivationFunctionType.Relu,
                            bias=act_bias_col[j], scale=-1.0)
                else:
                    op0 = (mybir.AluOpType.max if sp >= 0
                           else mybir.AluOpType.min)
                    nc.vector.tensor_scalar(
                        gt[:], uc[:], float(sp), float(sp),
                        op0, mybir.AluOpType.subtract)
                g.append(gt)

            # --- matmuls: one PSUM accumulation group, kt = 2*tile + ih ---
            ps = pspool.tile([O_LOC, B_LOC], F32, tag="ps")
            for kt in range(KT):
                t, ih = kt // 2, kt % 2
                rhs = g[t][:, ih * B_LOC:(ih + 1) * B_LOC]
                nc.tensor.matmul(ps[:], ckt[kt], rhs,
                                 start=(kt == 0), stop=(kt == KT - 1))

            # --- tail: out16 = ps + bias (ACT Identity, casts to fp16) ---
            out_sb = opool.tile([O_LOC, B_LOC], F16, tag="osb")
            nc.scalar.activation(
                out_sb[:], ps[:], mybir.ActivationFunctionType.Identity,
                bias=xb_sb[:, FW:FW + 1], scale=1.0)
            nc.sync.dma_start(out[:], out_sb[:])
    nc.compile()
    return nc


_NC_CACHE: dict = {}


def _get_nc(scale: float, ucbias: float) -> bass.Bass:
    key = (float(scale), float(ucbias))
    if key not in _NC_CACHE:
        _NC_CACHE[key] = _build_nc(scale, ucbias)
    return _NC_CACHE[key]


def prepare(x: np.ndarray, breakpoints: np.ndarray, values: np.ndarray):
    """Host prep: build the Bass graph (cached) + per-core input maps."""
    x = np.asarray(x, np.float32)
    breakpoints = np.asarray(breakpoints, np.float32)
    values = np.asarray(values, np.float32)

    # Grid affine params from the (shared) breakpoint row.
    bpr = breakpoints[0, 0].astype(np.float64)
    h = (bpr[-1] - bpr[0]) / S
    scale = float(1.0 / h)
    ucbias = float(-bpr[0] / h - SC)

    # Two-sided kink coefficients from the values.
    V = values.astype(np.float64)                    # [O, I, S+1]
    M = V[:, :, 1:] - V[:, :, :-1]                   # [O, I, S] slopes
    c = np.zeros((O, I, S))
    c[:, :, 1:] = M[:, :, 1:] - M[:, :, :-1]         # kinks at knots 1..19
    b_lin = M[:, :, 0] + c[:, :, 1:SC].sum(axis=2)   # affine part after
    a_tot = (V[:, :, 0] - (c[:, :, 1:SC] * np.arange(1, SC)).sum(axis=2)
             + b_lin * SC)                           # folding s'<0 kinks
    bias_o = a_tot.sum(axis=1)                       # [O]

    # Per-tile coefficient planes in production ORDER, sign per engine:
    #  tile j=0: uc            -> b_lin
    #  s'>0 (either engine)    -> +c_s    (basis relu(uc-s'))
    #  s'<0 on DVE             -> -c_s    (basis min(uc,s')-s')
    #  s'<0 on ACT             -> +c_s    (basis relu(s'-uc))
    planes = [b_lin]
    for sp, on_act in ORDER:
        s = sp + SC
        planes.append(c[:, :, s] if (sp >= 0 or on_act) else -c[:, :, s])
    Cf = np.stack(planes, axis=0).astype(np.float16)  # [NT, O, I]

    # Per-core layouts.
    Cr = Cf.reshape(NT, O_SPLIT, O_LOC, 2, 128)       # [t, oh, o, ih, j]
    x16 = x.astype(np.float16)
    xr = x16.reshape(B_SPLIT, B_LOC, 2, 128)          # [bq, b, ih, j]
    bias16 = bias_o.astype(np.float16).reshape(O_SPLIT, O_LOC)

    in_maps = []
    for core in range(N_CORES):
        bq, oh = core % B_SPLIT, core // B_SPLIT
        xb_c = np.zeros((128, XB_COLS), np.float16)
        # xr[bq] axes (b, ih, j) -> (j, ih, b)
        xb_c[:, 0:FW] = xr[bq].transpose(2, 1, 0).reshape(128, FW)
        xb_c[:, FW] = bias16[oh]
        # C: [j, kt = 2t+ih, o]
        C_c = np.ascontiguousarray(
            Cr[:, oh].transpose(3, 0, 2, 1)).reshape(128, KT * 128)
        in_maps.append({"xb": np.ascontiguousarray(xb_c), "C": C_c})

    nc = _get_nc(scale, ucbias)
    return nc, in_maps


def kernel(x: np.ndarray, breakpoints: np.ndarray, values: np.ndarray,
           **_extra) -> np.ndarray:
    nc, in_maps = prepare(x, breakpoints, values)
    res = run_bass_kernel_spmd(nc, in_maps, list(range(N_CORES)))

    outf = np.empty((B, O), np.float32)
    for core in range(N_CORES):
        bq, oh = core % B_SPLIT, core // B_SPLIT
        outf[bq * B_LOC:(bq + 1) * B_LOC, oh * O_LOC:(oh + 1) * O_LOC] = \
            res.results[core]["out"].T.astype(np.float32)
    return outf


if __name__ == "__main__":
    rng = np.random.default_rng(0)
    x = rng.uniform(-1, 1, (B, I)).astype(np.float32)
    bp = np.tile(np.linspace(-1, 1, S + 1, dtype=np.float32), (O, I, 1))
    v = (rng.standard_normal((O, I, S + 1)) * 0.1).astype(np.float32)
    out = kernel(x, bp, v)
    print("kernel ran, out:", out.shape, out.dtype, float(out.std()))
